# revision 1
# baseline (speedup 1.0000x reference)
"""Trainium2 Bass kernel for nn_MemoryBlock (scatter_memory).

Computes, for x [16384, 512] and memory [2000, 512]:
    logit = (x @ memory.T) / max(|x_row| * |mem_row|, 1e-8)   (cosine sim)
    w     = softmax(logit, axis=1)
    w     = w * (w > 1/2000)              (hard-shrink sparsify)
    w     = w / sum(w, axis=1)            (L1 renorm)
    out   = w @ memory
    loss  = 2e-4 * sum(-w * log(w + 1e-12))

Distribution: data-parallel across 8 NeuronCores (2048 batch rows each);
the [2000, 512] memory bank is replicated. The scalar loss is reduced on
host from per-core partials (no collectives needed).

Layout: on-chip tensors keep memory-index m on partitions and batch b on
the free axis, so the sparsified weights are already the transposed lhsT
the second matmul needs — zero on-chip transposes. Both matmul inputs are
fed pre-transposed from host (contraction dim on partitions).

Softmax reductions over m (the partition axis) use an all-ones stationary
fp32 matmul which also replicates the result across all partitions, so
subsequent elementwise ops need no partition broadcast.

Precision: the two big matmuls run in float32r (fp32 rounded to 12
mantissa bits, round-to-nearest — measured unbiased on HW); the softmax
denominator / L1-norm path is exact fp32 to keep hard-shrink threshold
flips rare. 1/|x| is folded into x on host; 1/|mem| rides the ACT exp's
per-partition scale operand; pad partitions get a -100 exp bias -> e=0.
"""

import contextlib

import numpy as np

import concourse.bacc as bacc
import concourse.tile as tile
from concourse import mybir
from concourse.bass_utils import run_bass_kernel_spmd

F32 = mybir.dt.float32
F32R = mybir.dt.float32r
ALU = mybir.AluOpType
ACTF = mybir.ActivationFunctionType

N_CORES = 8
B = 16384
F = 512
M = 2000
MP = 2048            # m padded to 16 tiles of 128
BC = B // N_CORES    # 2048 batch rows per core
BB = 512             # batch block (free-dim tile) per pipeline stage
NBB = BC // BB       # 4 blocks per core
KT = F // 128        # 4 contraction chunks
MT = MP // 128       # 16 memory tiles
BS = BB // 128       # 4 output row-tiles per block

THRESHOLD = 1.0 / M
EPS_COS = 1e-8
EPS_LOG = 1e-12
ENTROPY_COEF = 2e-4

_CACHE = {}


def _build(loop_n=None):
    nc = bacc.Bacc("TRN2", debug=False)

    # xt/memt/memn are declared float32r (same bytes as fp32) because they
    # feed fp32r matmuls. rmn is the per-memory-row 1/max-norm exp scale.
    xt = nc.dram_tensor("xt", [128, KT, BC], F32R, kind="ExternalInput")
    memt = nc.dram_tensor("memt", [128, KT, MP], F32R, kind="ExternalInput")
    memn = nc.dram_tensor("memn", [128, MT, F], F32R, kind="ExternalInput")
    rmn = nc.dram_tensor("rmn", [128, MT], F32, kind="ExternalInput")
    pbias = nc.dram_tensor("pbias", [128, 1], F32, kind="ExternalInput")
    zhat = nc.dram_tensor("zhat", [BC, F], F32, kind="ExternalOutput")
    ent = nc.dram_tensor("ent", [128, 1], F32, kind="ExternalOutput")

    with tile.TileContext(nc) as tc:
        with (
            tc.tile_pool(name="wgt", bufs=1) as wgt,
            tc.tile_pool(name="xtp", bufs=2) as xtp,
            tc.tile_pool(name="epool", bufs=18) as epool,
            tc.tile_pool(name="wpool", bufs=18) as wpool,
            tc.tile_pool(name="stats", bufs=2) as stats,
            tc.tile_pool(name="accs", bufs=2) as accs,
            tc.tile_pool(name="lgp", bufs=3) as lgp,
            tc.tile_pool(name="zsb", bufs=3) as zsb,
            tc.tile_pool(name="entp", bufs=1) as entp,
            tc.tile_pool(name="dot_ps", bufs=3, space="PSUM") as dot_ps,
            tc.tile_pool(name="stat_ps", bufs=1, space="PSUM") as stat_ps,
            tc.tile_pool(name="zh_ps", bufs=2, space="PSUM") as zh_ps,
        ):
            # Weights resident for the whole kernel (split per k/m chunk so
            # the first matmuls only wait on their own chunk's DMA).
            memt_sb = [
                wgt.tile([128, MP], F32R, tag=f"memt{k}", name=f"memt_sb{k}")
                for k in range(KT)
            ]
            for k in range(KT):
                nc.sync.dma_start(memt_sb[k][:], memt[:, k, :])
            memn_sb = [
                wgt.tile([128, F], F32R, tag=f"memn{mt}", name=f"memn_sb{mt}")
                for mt in range(MT)
            ]
            for mt in range(MT):
                nc.sync.dma_start(memn_sb[mt][:], memn[:, mt, :])
            rmn_sb = wgt.tile([128, MT], F32, tag="rmn")
            nc.sync.dma_start(rmn_sb[:], rmn[:])
            ones_sb = wgt.tile([128, 128], F32, tag="ones")
            nc.vector.memset(ones_sb[:], 1.0)
            eps_sb = wgt.tile([128, 1], F32, tag="epslog")
            nc.vector.memset(eps_sb[:], EPS_LOG)
            padbias = wgt.tile([128, 1], F32, tag="padbias")
            nc.sync.dma_start(padbias[:], pbias[:])

            ent_parts = entp.tile([128, NBB * MT], F32, tag="entparts")

            loop_cm = tc.For_i(0, loop_n, 1) if loop_n else contextlib.nullcontext()
            with loop_cm:
                for bb in range(NBB):
                    bsl = slice(bb * BB, (bb + 1) * BB)
                    xt_sb = [
                        xtp.tile([128, BB], F32R, tag=f"xt{k}", name=f"xt_sb{k}")
                        for k in range(KT)
                    ]
                    for k in range(KT):
                        nc.sync.dma_start(xt_sb[k][:], xt[:, k, bsl])

                    # dot -> e = exp(rmn_m * dot + padbias); e stays fp32.
                    e_tiles = []
                    for mt in range(MT):
                        dps = dot_ps.tile([128, BB], F32, tag="dot", name=f"d_{bb}_{mt}")
                        for k in range(KT):
                            nc.tensor.matmul(
                                dps[:],
                                memt_sb[k][:, mt * 128 : (mt + 1) * 128],
                                xt_sb[k][:],
                                start=(k == 0),
                                stop=(k == KT - 1),
                            )
                        e = epool.tile([128, BB], F32, tag="e", name=f"e_{bb}_{mt}")
                        nc.scalar.activation(
                            e[:], dps[:], ACTF.Exp,
                            bias=padbias[:] if mt == MT - 1 else 0.0,
                            scale=rmn_sb[:, mt : mt + 1],
                        )
                        e_tiles.append(e)

                    # den = sum_m e — exact fp32: serial accumulate + one fp32
                    # ones-matmul to reduce partitions & replicate everywhere.
                    dacc = accs.tile([128, BB], F32, tag="dacc")
                    nc.vector.tensor_add(dacc[:], e_tiles[0][:], e_tiles[1][:])
                    for mt in range(2, MT):
                        nc.vector.tensor_add(dacc[:], dacc[:], e_tiles[mt][:])
                    den_ps = stat_ps.tile([128, BB], F32, tag="den")
                    nc.tensor.matmul(den_ps[:], ones_sb[:], dacc[:], start=True, stop=True)
                    rden = stats.tile([128, BB], F32, tag="rden")
                    nc.vector.reciprocal(rden[:], den_ps[:])

                    # q = e * rden (softmax weight), then hard-shrink in place:
                    # q = (q > thr) * q
                    for mt in range(MT):
                        q = e_tiles[mt]
                        nc.vector.tensor_mul(q[:], q[:], rden[:])
                        nc.vector.scalar_tensor_tensor(
                            q[:], q[:], THRESHOLD, q[:], ALU.is_gt, ALU.mult
                        )

                    # L1 norm of sparsified q, exact fp32 path like den.
                    lacc = accs.tile([128, BB], F32, tag="lacc")
                    nc.vector.tensor_add(lacc[:], e_tiles[0][:], e_tiles[1][:])
                    for mt in range(2, MT):
                        nc.vector.tensor_add(lacc[:], lacc[:], e_tiles[mt][:])
                    l_ps = stat_ps.tile([128, BB], F32, tag="lsum")
                    nc.tensor.matmul(l_ps[:], ones_sb[:], lacc[:], start=True, stop=True)
                    rl = stats.tile([128, BB], F32, tag="rl")
                    nc.vector.reciprocal(rl[:], l_ps[:])

                    # Final weights (fp32r for the second matmul) + entropy.
                    w_tiles = []
                    for mt in range(MT):
                        w = wpool.tile([128, BB], F32R, tag="w", name=f"w_{bb}_{mt}")
                        nc.vector.tensor_mul(w[:], e_tiles[mt][:], rl[:])
                        w_tiles.append(w)
                        wf = w[:].bitcast(F32)
                        lgt = lgp.tile([128, BB], F32, tag="lg")
                        nc.scalar.activation(lgt[:], wf, ACTF.Ln, bias=eps_sb[:])
                        scr = lgp.tile([128, BB], F32, tag="scr")
                        nc.vector.scalar_tensor_tensor(
                            scr[:], wf, 1.0, lgt[:], ALU.mult, ALU.mult,
                            accum_out=ent_parts[:, bb * MT + mt : bb * MT + mt + 1],
                        )

                    # z_hat = w.T @ memn (contract over m), 128 batch rows/tile
                    for bs in range(BS):
                        zp = zh_ps.tile([128, F], F32, tag="zh", name=f"z_{bb}_{bs}")
                        for mt in range(MT):
                            nc.tensor.matmul(
                                zp[:],
                                w_tiles[mt][:, bs * 128 : (bs + 1) * 128],
                                memn_sb[mt][:],
                                start=(mt == 0),
                                stop=(mt == MT - 1),
                            )
                        zs = zsb.tile([128, F], F32, tag="zs")
                        nc.vector.tensor_copy(zs[:], zp[:])
                        nc.sync.dma_start(
                            zhat[bb * BB + bs * 128 : bb * BB + (bs + 1) * 128, :],
                            zs[:],
                        )

                ent_sb = entp.tile([128, 1], F32, tag="entout")
                nc.vector.reduce_sum(ent_sb[:], ent_parts[:], axis=mybir.AxisListType.X)
                nc.sync.dma_start(ent[:], ent_sb[:])

    nc.compile()
    return nc


def _prep_inputs(x: np.ndarray, memory: np.ndarray):
    """Shard/transpose/pad on host. Returns per-core input maps."""
    x = np.ascontiguousarray(x, dtype=np.float32).reshape(B, F)
    memory = np.ascontiguousarray(memory, dtype=np.float32)

    mn = np.linalg.norm(memory.astype(np.float32), axis=1)   # [M]
    xn = np.linalg.norm(x, axis=1)                           # [B]
    # Fold 1/|x| into x rows; 1/max(|x||mem|, eps) = rmn/|x| exactly when the
    # clamp is inactive (always true for this data: |x||mem| >> 1e-8).
    assert float((xn * mn.min()).min()) > 1e-6, "cosine eps clamp would bind"
    xh = (x / xn[:, None]).astype(np.float32)

    memt = np.zeros((F, MP), np.float32)
    memt[:, :M] = memory.T
    memt = np.ascontiguousarray(memt.reshape(KT, 128, MP).transpose(1, 0, 2))
    memn = np.zeros((MP, F), np.float32)
    memn[:M] = memory
    memn = np.ascontiguousarray(memn.reshape(MT, 128, F).transpose(1, 0, 2))
    rmn_pad = np.zeros(MP, np.float32)
    rmn_pad[:M] = 1.0 / mn
    rmn_pad = np.ascontiguousarray(rmn_pad.reshape(MT, 128).T)  # [128, MT]
    pbias_np = np.zeros((128, 1), np.float32)
    pbias_np[M - (MT - 1) * 128 :] = -100.0

    in_maps = []
    for c in range(N_CORES):
        xs = xh[c * BC : (c + 1) * BC]            # [BC, F]
        xtc = np.ascontiguousarray(xs.T.reshape(KT, 128, BC).transpose(1, 0, 2))
        in_maps.append({"xt": xtc, "memt": memt, "memn": memn, "rmn": rmn_pad,
                        "pbias": pbias_np})
    return in_maps


def kernel(x: np.ndarray, memory: np.ndarray, _trace: bool = False, **_ignored):
    if "nc" not in _CACHE:
        _CACHE["nc"] = _build()
    nc = _CACHE["nc"]

    in_maps = _prep_inputs(x, memory)
    res = run_bass_kernel_spmd(nc, in_maps, list(range(N_CORES)), trace=_trace)
    _CACHE["last_result"] = res

    out = np.empty((B, F), np.float32)
    ent_total = 0.0
    for c in range(N_CORES):
        out[c * BC : (c + 1) * BC] = res.results[c]["zhat"]
        ent_total += float(res.results[c]["ent"].sum())
    mem_loss = np.float32(-ENTROPY_COEF * ent_total)
    return out.reshape(x.shape), mem_loss
